# revision 5
# baseline (speedup 1.0000x reference)
"""BiMamba Trainium2 kernel, v3.

Sharding: each of the 8 cores owns a 256-channel slice of d_inner for BOTH
directions (fwd+rev share in_proj/out_proj; the reversed direction's in_proj
output is a flipped view of the forward one).

v3 (vs v2): fp16 activation path end-to-end (conv/x_proj/dt/scan/out_proj),
fp16 collectives (AllReduce of x_dbl partials, 4-way chunked ReduceScatter
of out_proj partials), B/C broadcast directly from the AllReduce result in
DRAM (no cast staging), chunked conv for head latency, hC mults mostly on
GpSimd, n-state reduction via identity-matmul accumulation in PSUM with
D*x as the init term and silu(z) gating fused into the PSUM eviction, one
fused out_proj (fwd+rev in one accumulation group) feeding the chunked RS.
"""

import os
import sys

sys.path.insert(0, "/opt/trn_rl_repo")

import numpy as np
import ml_dtypes

# ---------------------------------------------------------------- constants
P = 128           # partitions
L = 2048          # sequence length
DM = 1024         # d_model
DI = 2048         # d_inner
NST = 16          # d_state
RK = 64           # dt_rank
KCONV = 4         # conv width
NCORES = 8
CH = DI // NCORES          # channels per core per direction = 256
NPT = CH // P              # channel ptiles per core = 2
FB = 512                   # matmul moving free chunk
NFB = L // FB              # 4
PAD = KCONV - 1            # causal pad = 3
NXP = RK + 2 * NST         # 96
NRS = 4                    # ReduceScatter chunks
RSL = L // NRS             # 512 rows per RS chunk
RSO = RSL // NCORES        # 64 rows per core per RS chunk


def build_program(num_cores=NCORES, enable_asserts=False):
    import concourse.bass as bass
    import concourse.mybir as mybir
    import concourse.tile as tile
    from concourse import bacc
    from contextlib import ExitStack

    dt = mybir.dt
    AF = mybir.ActivationFunctionType
    OP = mybir.AluOpType
    F16 = dt.float16

    nc = bacc.Bacc(
        "TRN2",
        target_bir_lowering=False,
        debug=False,
        enable_asserts=enable_asserts,
        num_devices=num_cores,
    )

    # ------------------------------------------------------------- dram I/O
    hidden = nc.dram_tensor("hidden", [L, DM], dt.float32, kind="ExternalInput")
    w_inT = nc.dram_tensor("w_inT", [DM, 2 * CH], dt.bfloat16, kind="ExternalInput")
    w_outT = nc.dram_tensor("w_outT", [CH, DM], F16, kind="ExternalInput")
    w_xT = {}
    w_dtT = {}
    conv_w = {}
    conv_b = {}
    dt_b = {}
    A_in = {}
    D_in = {}
    for d in ("f", "r"):
        w_xT[d] = nc.dram_tensor(f"w_xT_{d}", [CH, NXP], F16,
                                 kind="ExternalInput")
        w_dtT[d] = nc.dram_tensor(f"w_dtT_{d}", [RK, CH], F16,
                                  kind="ExternalInput")
        conv_w[d] = nc.dram_tensor(f"conv_w_{d}", [CH, KCONV], dt.float32,
                                   kind="ExternalInput")
        conv_b[d] = nc.dram_tensor(f"conv_b_{d}", [CH, 1], dt.float32,
                                   kind="ExternalInput")
        dt_b[d] = nc.dram_tensor(f"dt_b_{d}", [CH, 1], dt.float32,
                                 kind="ExternalInput")
        A_in[d] = nc.dram_tensor(f"A_{d}", [CH, NST], dt.float32,
                                 kind="ExternalInput")
        D_in[d] = nc.dram_tensor(f"D_{d}", [CH, 1], dt.float32,
                                 kind="ExternalInput")
    ident = nc.dram_tensor("ident", [P, P], dt.float32, kind="ExternalInput")
    out = nc.dram_tensor("out", [NRS * RSO, DM], F16, kind="ExternalOutput")

    NKB = DM // P  # 8
    NTT = L // P   # 16
    WPAD = L + 2 * PAD

    with tile.TileContext(nc) as tc:
        ctx = ExitStack()
        with ctx:
            dram = ctx.enter_context(tc.tile_pool(name="dram", bufs=1, space="DRAM"))
            consts = ctx.enter_context(tc.tile_pool(name="consts", bufs=1))
            psum_mm = ctx.enter_context(
                tc.tile_pool(name="psum_mm", bufs=3, space="PSUM"))

            # ---------------------------------------------------- constants
            ident_sb = consts.tile([P, P], dt.float32)
            nc.sync.dma_start(ident_sb[:], ident[:])
            ident16 = consts.tile([P, P], F16, name="ident16", tag="ident16")
            nc.vector.tensor_copy(ident16[:], ident_sb[:])
            conv_w_sb = {}
            conv_b_sb = {}
            dt_b_sb = {}
            A_sb = {}
            D_sb = {}
            for d in ("f", "r"):
                for pb in range(NPT):
                    ps = slice(pb * P, (pb + 1) * P)
                    for nm, store, src, shape in (
                        ("cw", conv_w_sb, conv_w, [P, KCONV]),
                        ("cb", conv_b_sb, conv_b, [P, 1]),
                        ("db", dt_b_sb, dt_b, [P, 1]),
                        ("A", A_sb, A_in, [P, NST]),
                        ("Dc", D_sb, D_in, [P, 1]),
                    ):
                        t = consts.tile(shape, dt.float32, name=f"{nm}{d}{pb}",
                                        tag=f"{nm}{d}{pb}")
                        nc.sync.dma_start(t[:], src[d][ps, :])
                        store[d, pb] = t
            w_dt_sb = {}
            for d in ("f", "r"):
                w_dt_sb[d] = consts.tile([RK, CH], F16, name=f"wdt{d}",
                                         tag=f"wdt{d}")
                nc.sync.dma_start(w_dt_sb[d][:], w_dtT[d][:])
            w_x_sb = {}
            for d in ("f", "r"):
                for pb in range(NPT):
                    t = consts.tile([P, NXP], F16, name=f"wx{d}{pb}",
                                    tag=f"wx{d}{pb}")
                    nc.sync.dma_start(t[:], w_xT[d][pb * P:(pb + 1) * P, :])
                    w_x_sb[d, pb] = t
            w_out_sb = []
            for pb in range(NPT):
                t = consts.tile([P, DM], F16, name=f"wo{pb}", tag=f"wo{pb}")
                nc.sync.dma_start(t[:], w_outT[pb * P:(pb + 1) * P, :])
                w_out_sb.append(t)

            # persistent activation buffers (gz = silu(z), forward time; both
            # directions gate with it — the rev pipeline un-flips at the tree)
            gz_pool = ctx.enter_context(tc.tile_pool(name="gzp", bufs=1))
            gz = {}
            for pb in range(NPT):
                gz[pb] = gz_pool.tile([P, L], F16, name=f"gz{pb}", tag=f"gz{pb}")
            dt_pool = ctx.enter_context(tc.tile_pool(name="dtp", bufs=2))
            dtx_pool = ctx.enter_context(tc.tile_pool(name="dtxp", bufs=2))
            dxc_pool = ctx.enter_context(tc.tile_pool(name="dxcp", bufs=2))
            xc_pool = ctx.enter_context(tc.tile_pool(name="xcp", bufs=4))

            xdbl_part = dram.tile([2 * NXP, L], F16)
            xdbl_sum = {di: dram.tile([NXP, L], F16, addr_space="Shared",
                                      name=f"xsum{di}", tag=f"xsum{di}")
                        for di in range(2)}
            pout = dram.tile([L, DM], F16)
            pout_rs = dram.tile([NRS * RSO, DM], F16)
            pf_dram = dram.tile([L, DM], F16, name="pfd", tag="pfd")

            # stage-limited pools
            ctxB = ExitStack()
            xpad_pool = ctxB.enter_context(tc.tile_pool(name="xpadp", bufs=1))
            cacc_pool = ctxB.enter_context(tc.tile_pool(name="caccp", bufs=6))
            xev_pool = ctxB.enter_context(tc.tile_pool(name="xevp", bufs=2))
            ctxA = ExitStack()
            hT_pool = ctxA.enter_context(tc.tile_pool(name="hTp", bufs=1))
            hnat_pool = ctxA.enter_context(tc.tile_pool(name="hnatp", bufs=4))
            w_in_pool = ctxA.enter_context(tc.tile_pool(name="winp", bufs=1))

            # ---- head: transpose + in_proj + conv_f + xproj_f, chunk-pipelined
            psum_tp = ctxA.enter_context(
                tc.tile_pool(name="psum_tp", bufs=3, space="PSUM"))
            hT = [hT_pool.tile([P, L], dt.bfloat16, name=f"hT{k}", tag=f"hT{k}")
                  for k in range(NKB)]
            w_in_sb = [w_in_pool.tile([P, 2 * CH], dt.bfloat16, name=f"win{k}",
                                      tag=f"win{k}") for k in range(NKB)]
            for k in range(NKB):
                nc.sync.dma_start(w_in_sb[k][:], w_inT[k * P:(k + 1) * P, :])

            xpad = [xpad_pool.tile([P, WPAD], F16, name=f"xpad{pb}",
                                   tag=f"xpad{pb}") for pb in range(NPT)]
            for pb in range(NPT):
                nc.vector.memset(xpad[pb][:, 0:PAD], 0.0)
                nc.vector.memset(xpad[pb][:, PAD + L:WPAD], 0.0)

            xc = {}
            for d in ("f", "r"):
                for pb in range(NPT):
                    xc[d, pb] = xc_pool.tile([P, L], F16, name=f"xc{d}{pb}",
                                             tag=f"xc{d}{pb}")

            def conv_chunk(d, pb, c):
                cw = conv_w_sb[d, pb]
                cb = conv_b_sb[d, pb]
                sl = slice(c * FB, (c + 1) * FB)
                if d == "f":
                    taps = [xpad[pb][:, k + c * FB:k + (c + 1) * FB]
                            for k in range(KCONV)]
                else:
                    taps = [xpad[pb][:, 2 * PAD - k: 2 * PAD - k + L]
                            [:, ::-1][:, sl] for k in range(KCONV)]
                acc = cacc_pool.tile([P, FB], F16, name="cacc", tag="cacc")
                nc.scalar.activation(acc[:], taps[0], AF.Identity,
                                     bias=cb[:, 0:1], scale=cw[:, 0:1])
                for k in range(1, KCONV):
                    acc2 = cacc_pool.tile([P, FB], F16, name="cacc", tag="cacc")
                    nc.vector.scalar_tensor_tensor(
                        acc2[:], taps[k], cw[:, k:k + 1], acc[:],
                        OP.mult, OP.add)
                    acc = acc2
                nc.scalar.activation(xc[d, pb][:, sl], acc[:], AF.Silu)

            def xproj_chunk(d, di, fb):
                pm = psum_mm.tile([NXP, FB], dt.float32, name="mmx", tag="mm")
                for pb in range(NPT):
                    nc.tensor.matmul(
                        pm[:],
                        w_x_sb[d, pb][:],
                        xc[d, pb][:, fb * FB:(fb + 1) * FB],
                        start=(pb == 0),
                        stop=(pb == NPT - 1),
                    )
                xev = xev_pool.tile([NXP, FB], F16, name="xev", tag="xev")
                nc.scalar.copy(xev[:], pm[:])
                nc.sync.dma_start(
                    xdbl_part[di * NXP:(di + 1) * NXP, fb * FB:(fb + 1) * FB],
                    xev[:])

            for q in range(NTT // 4):  # q doubles as the fb chunk index
                hn = []
                for j in range(4):
                    t = hnat_pool.tile([P, DM], dt.float32, name="hnat", tag="hnat")
                    nc.sync.dma_start(
                        t[:], hidden[(q * 4 + j) * P:(q * 4 + j + 1) * P, :])
                    hn.append(t)
                for kb in range(NKB):
                    pt = psum_tp.tile([P, 4 * P], dt.float32, name="tp", tag="tp")
                    for j in range(4):
                        nc.tensor.transpose(
                            pt[:, j * P:(j + 1) * P],
                            hn[j][:, kb * P:(kb + 1) * P],
                            ident_sb[:],
                        )
                    nc.scalar.copy(hT[kb][:, q * 4 * P:(q + 1) * 4 * P], pt[:])
                fb = q
                for mb in range(NPT):  # x half only (z deferred past the AR)
                    pm = psum_mm.tile([P, FB], dt.float32, name="mm", tag="mm")
                    for k in range(NKB):
                        nc.tensor.matmul(
                            pm[:],
                            w_in_sb[k][:, mb * P:(mb + 1) * P],
                            hT[k][:, fb * FB:(fb + 1) * FB],
                            start=(k == 0),
                            stop=(k == NKB - 1),
                        )
                    nc.scalar.copy(
                        xpad[mb][:, PAD + fb * FB: PAD + (fb + 1) * FB], pm[:])
                for pb in range(NPT):
                    conv_chunk("f", pb, fb)
                xproj_chunk("f", 0, fb)

            nc.gpsimd.collective_compute(
                "AllReduce",
                OP.add,
                replica_groups=[list(range(num_cores))],
                ins=[xdbl_part[0:NXP, :].opt()],
                outs=[xdbl_sum[0][:].opt()],
            )

            for c in range(NFB):
                for pb in range(NPT):
                    conv_chunk("r", pb, c)
                xproj_chunk("r", 1, c)
            nc.gpsimd.collective_compute(
                "AllReduce",
                OP.add,
                replica_groups=[list(range(num_cores))],
                ins=[xdbl_part[NXP:2 * NXP, :].opt()],
                outs=[xdbl_sum[1][:].opt()],
            )

            # deferred z half of in_proj -> silu(z), fills the AR latency
            for fb in range(NFB):
                for mb in range(NPT, 2 * NPT):
                    pm = psum_mm.tile([P, FB], dt.float32, name="mm", tag="mm")
                    for k in range(NKB):
                        nc.tensor.matmul(
                            pm[:],
                            w_in_sb[k][:, mb * P:(mb + 1) * P],
                            hT[k][:, fb * FB:(fb + 1) * FB],
                            start=(k == 0),
                            stop=(k == NKB - 1),
                        )
                    pb = mb - NPT
                    nc.scalar.activation(
                        gz[pb][:, fb * FB:(fb + 1) * FB], pm[:], AF.Silu)
            ctxA.close()
            ctxB.close()
            y_pool = ctx.enter_context(tc.tile_pool(name="yp", bufs=4))
            oev_pool = ctx.enter_context(tc.tile_pool(name="oevp", bufs=3))
            xdbl_pool = ctx.enter_context(tc.tile_pool(name="xdblp", bufs=1))
            etmp_pool = ctx.enter_context(tc.tile_pool(name="etmpp", bufs=1))
            pf_pool = ctx.enter_context(tc.tile_pool(name="pfp", bufs=2))

            # --------------------------------- dt stage (per direction)
            dt_sb = {}
            dtx = {}
            dxc = {}

            def dt_block(d, di):
                xdbl = xdbl_pool.tile([NXP, L], F16, name="xdbl", tag="xdbl")
                nc.sync.dma_start(xdbl[:], xdbl_sum[di][:])
                for pb in range(NPT):
                    t = dt_pool.tile([P, L], F16, name="dtt", tag="dtt")
                    for fb in range(NFB):
                        pm = psum_mm.tile([P, FB], dt.float32, name="mm", tag="mm")
                        nc.tensor.matmul(
                            pm[:],
                            w_dt_sb[d][:, pb * P:(pb + 1) * P],
                            xdbl[0:RK, fb * FB:(fb + 1) * FB],
                            start=True, stop=True)
                        et = etmp_pool.tile([P, FB], dt.float32, name="etmp",
                                            tag="etmp")
                        nc.scalar.activation(
                            et[:], pm[:], AF.Exp, bias=dt_b_sb[d, pb][:, 0:1])
                        nc.scalar.activation(
                            t[:, fb * FB:(fb + 1) * FB], et[:], AF.Ln, bias=1.0)
                    dt_sb[d, pb] = t
                    tx = dtx_pool.tile([P, L], F16, name="dtx", tag="dtx")
                    nc.vector.tensor_mul(tx[:], t[:], xc[d, pb][:])
                    dtx[d, pb] = tx
                    dc = dxc_pool.tile([P, L], F16, name="dxc", tag="dxc")
                    nc.vector.tensor_scalar_mul(
                        dc[:], xc[d, pb][:], D_sb[d, pb][:, 0:1])
                    dxc[d, pb] = dc

            dt_block("f", 0)

            # --------------------------------- scan passes
            psum_tree = ctx.enter_context(
                tc.tile_pool(name="psum_tree", bufs=1, space="PSUM"))
            bbc_pool = ctx.enter_context(tc.tile_pool(name="bbcp", bufs=2))
            cbc_pool = ctx.enter_context(tc.tile_pool(name="cbcp", bufs=2))
            da_pool = ctx.enter_context(tc.tile_pool(name="dap", bufs=2))
            dbx_pool = ctx.enter_context(tc.tile_pool(name="dbxp", bufs=1))
            h_pool = ctx.enter_context(tc.tile_pool(name="hp", bufs=1))
            hc_pool = ctx.enter_context(tc.tile_pool(name="hcp", bufs=2))

            y = {}
            NPAIR = NST // 2

            def scan_pass(d, pb, di):
                """One (direction, ptile) pass; two states per scan
                instruction (the recurrence is reset at the seam by zeroing
                the first da column of each state, exact since h starts at
                0).  For d == 'r' the internal time axis is reversed and the
                PE tree un-flips via reversed views."""
                rev = d == "r"
                tree = psum_tree.tile([P, L], dt.float32, name="tree", tag="tree")
                dxv = dxc[d, pb][:, ::-1] if rev else dxc[d, pb][:]
                for c in range(NFB):
                    sl = slice(c * FB, (c + 1) * FB)
                    nc.tensor.matmul(tree[:, sl], ident16[:], dxv[:, sl],
                                     start=True, stop=False)
                for pr in range(NPAIR):
                    n0 = 2 * pr
                    rb = xdbl_sum[di][RK + n0:RK + n0 + 2, :]
                    bb = bbc_pool.tile([P, 2 * L], F16, name="bbc", tag="bbc")
                    nc.sync.dma_start(
                        bb[:], bass.AP(rb.tensor, rb.offset, [[0, P], [1, 2 * L]]))
                    rc = xdbl_sum[di][RK + NST + n0:RK + NST + n0 + 2, :]
                    cbt = cbc_pool.tile([P, 2 * L], F16, name="cbc", tag="cbc")
                    nc.scalar.dma_start(
                        cbt[:], bass.AP(rc.tensor, rc.offset, [[0, P], [1, 2 * L]]))
                    da = da_pool.tile([P, 2 * L], F16, name="da", tag="da")
                    for sL in range(2):
                        nc.scalar.activation(
                            da[:, sL * L:(sL + 1) * L], dt_sb[d, pb][:], AF.Exp,
                            scale=A_sb[d, pb][:, n0 + sL:n0 + sL + 1])
                    nc.vector.memset(da[:, 0:L + 1:L], 0.0)
                    dbx = dbx_pool.tile([P, 2 * L], F16, name="dbx", tag="dbx")
                    for sL in range(2):
                        nc.vector.tensor_mul(
                            dbx[:, sL * L:(sL + 1) * L], dtx[d, pb][:],
                            bb[:, sL * L:(sL + 1) * L])
                    h = h_pool.tile([P, 2 * L], F16, name="h", tag="h")
                    nc.vector.tensor_tensor_scan(
                        h[:], da[:], dbx[:], 0.0, OP.mult, OP.add)
                    for sL in range(2):
                        hc = hc_pool.tile([P, L], F16, name="hc", tag="hc")
                        nc.vector.tensor_mul(
                            hc[:], h[:, sL * L:(sL + 1) * L],
                            cbt[:, sL * L:(sL + 1) * L])
                        hcv = hc[:, ::-1] if rev else hc[:]
                        last = pr == NPAIR - 1 and sL == 1
                        for c in range(NFB):
                            sl = slice(c * FB, (c + 1) * FB)
                            nc.tensor.matmul(tree[:, sl], ident16[:],
                                             hcv[:, sl],
                                             start=False, stop=last)
                yt = y_pool.tile([P, L], F16, name="y", tag="y")
                for c in range(NFB):
                    sl = slice(c * FB, (c + 1) * FB)
                    nc.vector.tensor_mul(yt[:, sl], tree[:, sl], gz[pb][:, sl])
                y[d, pb] = yt

            for pb in range(NPT):
                scan_pass("f", pb, 0)

            dt_block("r", 1)

            # out_proj for fwd -> DRAM fp16 stash (overlaps the rev passes)
            for tb in range(NTT):
                tsl = slice(tb * P, (tb + 1) * P)
                for fb in range(DM // FB):
                    pm = psum_mm.tile([P, FB], dt.float32, name="mmo", tag="mm")
                    for pb in range(NPT):
                        nc.tensor.matmul(
                            pm[:],
                            y["f", pb][:, tsl],
                            w_out_sb[pb][:, fb * FB:(fb + 1) * FB],
                            start=(pb == 0),
                            stop=(pb == NPT - 1),
                        )
                    pf = pf_pool.tile([P, FB], F16, name="pf", tag="pf")
                    nc.scalar.copy(pf[:], pm[:])
                    nc.sync.dma_start(
                        pf_dram[tsl, fb * FB:(fb + 1) * FB], pf[:])

            for pb in range(NPT):
                scan_pass("r", pb, 1)

            # ---- out_proj rev + fwd stash add-back + chunked ReduceScatter
            for g in range(NRS):
                for tbl in range(NTT // NRS):
                    tb = g * (NTT // NRS) + tbl
                    tsl = slice(tb * P, (tb + 1) * P)
                    for fb in range(DM // FB):
                        pfb = pf_pool.tile([P, FB], F16, name="pfb", tag="pf")
                        nc.sync.dma_start(
                            pfb[:], pf_dram[tsl, fb * FB:(fb + 1) * FB])
                        pm = psum_mm.tile([P, FB], dt.float32, name="mmo",
                                          tag="mm")
                        for pb in range(NPT):
                            nc.tensor.matmul(
                                pm[:],
                                y["r", pb][:, tsl],
                                w_out_sb[pb][:, fb * FB:(fb + 1) * FB],
                                start=(pb == 0), stop=False)
                        nc.tensor.matmul(
                            pm[:], ident16[:], pfb[:],
                            start=False, stop=True)
                        oev = oev_pool.tile([P, FB], F16, name="oev", tag="oev")
                        nc.scalar.copy(oev[:], pm[:])
                        nc.sync.dma_start(
                            pout[tsl, fb * FB:(fb + 1) * FB], oev[:])
                nc.gpsimd.collective_compute(
                    "ReduceScatter",
                    OP.add,
                    replica_groups=[list(range(num_cores))],
                    ins=[pout[g * RSL:(g + 1) * RSL, :].opt()],
                    outs=[pout_rs[g * RSO:(g + 1) * RSO, :].opt()],
                )
            nc.sync.dma_start(out[:], pout_rs[:])

    return nc


# ---------------------------------------------------------------- host side
def _make_in_maps(inputs):
    h = np.ascontiguousarray(np.asarray(inputs["hidden_states"],
                                        dtype=np.float32).reshape(L, DM))
    w_in = np.asarray(inputs["in_proj_w"], dtype=np.float32)     # (2DI, DM)
    w_out = np.asarray(inputs["out_proj_w"], dtype=np.float32)   # (DM, DI)
    ident = np.eye(P, dtype=np.float32)

    in_maps = []
    for c in range(NCORES):
        sl = slice(c * CH, (c + 1) * CH)
        m = {"hidden": h, "ident": ident}
        w_slice = np.concatenate(
            [w_in[sl, :], w_in[DI + c * CH: DI + (c + 1) * CH, :]], axis=0)
        m["w_inT"] = np.ascontiguousarray(
            w_slice.T).astype(ml_dtypes.bfloat16)
        m["w_outT"] = np.ascontiguousarray(
            w_out[:, sl].T).astype(np.float16)
        for d, tag in (("f", "_f"), ("r", "_r")):
            w_x = np.asarray(inputs[f"x_proj_w{tag}"], dtype=np.float32)
            m[f"w_xT_{d}"] = np.ascontiguousarray(
                w_x[:, sl].T).astype(np.float16)
            w_dt = np.asarray(inputs[f"dt_proj_w{tag}"], dtype=np.float32)
            m[f"w_dtT_{d}"] = np.ascontiguousarray(
                w_dt[sl, :].T).astype(np.float16)
            m[f"conv_w_{d}"] = np.ascontiguousarray(
                np.asarray(inputs[f"conv_w{tag}"], dtype=np.float32)[sl, :])
            m[f"conv_b_{d}"] = np.ascontiguousarray(
                np.asarray(inputs[f"conv_b{tag}"], dtype=np.float32)[sl, None])
            m[f"dt_b_{d}"] = np.ascontiguousarray(
                np.asarray(inputs[f"dt_proj_b{tag}"], dtype=np.float32)[sl, None])
            m[f"A_{d}"] = np.ascontiguousarray(
                -np.exp(np.asarray(inputs[f"A_log{tag}"], dtype=np.float32)[sl, :]))
            m[f"D_{d}"] = np.ascontiguousarray(
                np.asarray(inputs[f"D{tag}"], dtype=np.float32)[sl, None])
        in_maps.append(m)
    return in_maps


_CACHED = {}


def _install_ntff_hook_shim():
    import types
    try:
        import antenv.axon_hooks  # noqa: F401
        return
    except ImportError:
        pass
    import antenv
    mod = types.ModuleType("antenv.axon_hooks")
    _state = {"h": None}
    mod.get_axon_ntff_profile_hook = lambda: _state["h"]
    mod.set_axon_ntff_profile_hook = lambda h: _state.__setitem__("h", h)
    sys.modules["antenv.axon_hooks"] = mod
    antenv.axon_hooks = mod
    try:
        from trn_agent_boot.trn_boot import _ntff_profile_via_ctypes
        hook = _ntff_profile_via_ctypes("/opt/axon/libaxon_pjrt.so")
        if hook is not None:
            mod.set_axon_ntff_profile_hook(hook)
    except Exception:
        pass


def _install_hook_err_capture():
    import traceback
    import concourse.bass2jax as b2j
    if getattr(b2j, "_err_capture_installed", False):
        return
    orig = b2j.neuronx_cc_hook

    def wrapped(*a):
        try:
            return orig(*a)
        except Exception:
            with open("/tmp/hook_err.log", "w") as f:
                f.write(traceback.format_exc())
            raise

    b2j.neuronx_cc_hook = wrapped
    b2j._err_capture_installed = True


def kernel(**inputs):
    from concourse.bass_utils import run_bass_kernel_spmd

    _install_ntff_hook_shim()
    _install_hook_err_capture()

    if "nc" not in _CACHED:
        from concourse.bass_interp import get_hw_module
        nc = build_program()
        nc.finalize()
        nc.m = get_hw_module(nc.m)
        _CACHED["nc"] = nc
    nc = _CACHED["nc"]

    in_maps = _make_in_maps(inputs)
    res = run_bass_kernel_spmd(
        nc, in_maps, core_ids=list(range(NCORES)),
        trace=bool(int(os.environ.get("KERNEL_TRACE", "0"))),
    )
    _CACHED["last_result"] = res
    # core c's 'out' holds rows {g*RSL + c*RSO + i} for g in 0..NRS-1
    full = np.empty((L, DM), dtype=np.float32)
    for c in range(NCORES):
        oc = np.asarray(res.results[c]["out"], dtype=np.float32)
        for g in range(NRS):
            full[g * RSL + c * RSO: g * RSL + (c + 1) * RSO, :] = \
                oc[g * RSO:(g + 1) * RSO, :]
    return full.reshape(1, L, DM)


if __name__ == "__main__":
    nc = build_program()
    try:
        n = sum(len(bb.instructions) for bb in nc.main_func.blocks)
    except Exception:
        n = "?"
    print("build ok; instructions:", n)
